# revision 1
# baseline (speedup 1.0000x reference)
"""Multi-head self-attention (B=4, S=2048, D=512, H=8) on 8 trn2 NeuronCores.

Sharding: core c -> (batch c//2, heads 4*(c%2) .. 4*(c%2)+3)  [batch x head-half].
Each core computes a partial transposed output finalT_c [D, S] =
Wo[:, head_slice] @ ctx_heads.T ; host sums the two partials per batch,
transposes back and adds bo.
"""

import sys

sys.path.insert(0, "/opt/trn_rl_repo")

import functools
from contextlib import ExitStack

import numpy as np

B, S, D, H = 4, 2048, 512, 8
DK = D // H           # 64
HLOC = H // 2         # 4 heads per core
DH = HLOC * DK        # 256 local head dims
ST = S // 128         # 16 s(k) tiles
DT = D // 128         # 4 din tiles
NCH = S // 512        # 4 free-dim chunks of 512
QH = 2                # q halves of 1024
SCALE = 1.0 / float(np.sqrt(DK))


def _build(reps=1):
    import contextlib
    import concourse.tile as tile
    from concourse import bacc, mybir

    f32 = mybir.dt.float32
    f32r = mybir.dt.float32r
    r = lambda ap: ap.bitcast(f32r)  # full-rate PE matmul mode
    nc = bacc.Bacc("TRN2", target_bir_lowering=False, debug=False, num_devices=8)

    xT = nc.dram_tensor("xT", [D, S], f32r, kind="ExternalInput").ap()
    wqT = nc.dram_tensor("wqT", [D, DH], f32r, kind="ExternalInput").ap()
    wkT = nc.dram_tensor("wkT", [D, DH], f32r, kind="ExternalInput").ap()
    wvT = nc.dram_tensor("wvT", [D, DH], f32r, kind="ExternalInput").ap()
    woT = nc.dram_tensor("woT", [DH, D], f32r, kind="ExternalInput").ap()
    bqv = nc.dram_tensor("bq", [DH], f32, kind="ExternalInput").ap()
    bkv = nc.dram_tensor("bk", [DH], f32, kind="ExternalInput").ap()
    bvv = nc.dram_tensor("bv", [DH], f32r, kind="ExternalInput").ap()
    maskb = nc.dram_tensor("maskb", [S], f32, kind="ExternalInput").ap()
    outT = nc.dram_tensor("outT", [D, S], f32, kind="ExternalOutput").ap()

    with tile.TileContext(nc) as tc, ExitStack() as ctx:
        Exp = mybir.ActivationFunctionType.Exp

        consts = ctx.enter_context(tc.tile_pool(name="consts", bufs=1))
        xpool = ctx.enter_context(tc.tile_pool(name="xpool", bufs=1))
        qkpool = ctx.enter_context(tc.tile_pool(name="qkpool", bufs=1))
        vpool = ctx.enter_context(tc.tile_pool(name="vpool", bufs=1))
        cpool = ctx.enter_context(tc.tile_pool(name="cpool", bufs=1))
        eppool = ctx.enter_context(tc.tile_pool(name="eppool", bufs=4))
        nrmpool = ctx.enter_context(tc.tile_pool(name="nrmpool", bufs=2))
        outpool = ctx.enter_context(tc.tile_pool(name="outpool", bufs=2))

        # ---- loads -------------------------------------------------------
        def load(pool, dram_ap, shape, tag, dt=None):
            t = pool.tile(shape, dt or f32, tag=tag, name=tag)
            nc.sync.dma_start(out=t[:], in_=dram_ap)
            return t

        wq = [load(consts, wqT[128 * t : 128 * (t + 1), :], [128, DH], f"wq{t}", f32r)
              for t in range(DT)]
        xt = [load(xpool, xT[128 * t : 128 * (t + 1), :], [128, S], f"xt{t}", f32r)
              for t in range(DT)]
        wk = [load(consts, wkT[128 * t : 128 * (t + 1), :], [128, DH], f"wk{t}", f32r)
              for t in range(DT)]
        wv = [load(consts, wvT[128 * t : 128 * (t + 1), :], [128, DH], f"wv{t}", f32r)
              for t in range(DT)]
        wo = [load(consts, woT[128 * t : 128 * (t + 1), :], [128, D], f"wo{t}", f32r)
              for t in range(2)]
        bq_sb = load(consts, bqv.rearrange("(m p) -> p m", p=128), [128, 2], "bq")
        bk_sb = load(consts, bkv.rearrange("(m p) -> p m", p=128), [128, 2], "bk")
        bv_sb = load(consts, bvv.rearrange("(o d) -> o d", o=1), [1, DH], "bv", f32r)
        mk_sb = load(consts, maskb.rearrange("(k p) -> p k", p=128), [128, ST], "mk")

        ones_row = consts.tile([1, 128], f32r, tag="ones", name="ones")
        nc.vector.memset(ones_row[:].bitcast(f32), 1.0)  # memset ISA lacks f32r

        # reps>1 wraps the compute in an on-device loop (benchmarking only)
        rep_ctx = tc.For_i(0, reps, 1) if reps > 1 else contextlib.nullcontext()
        ctx.enter_context(rep_ctx)

        # ---- projections -------------------------------------------------
        with tc.tile_pool(name="pproj", bufs=2, space="PSUM") as ppool:
            qt, kt = [], []
            for w_tiles, bias, dst in ((wq, bq_sb, qt), (wk, bk_sb, kt)):
                for m in range(2):
                    ps = ppool.tile([128, S], f32, tag="ps", name="ps")
                    for t in range(DT):
                        for c in range(NCH):
                            nc.tensor.matmul(
                                ps[:, 512 * c : 512 * (c + 1)],
                                lhsT=(w_tiles[t][:, 128 * m : 128 * (m + 1)]),
                                rhs=(xt[t][:, 512 * c : 512 * (c + 1)]),
                                start=(t == 0), stop=(t == DT - 1),
                            )
                    sb = qkpool.tile([128, S], f32r, tag=f"qk{len(qt) + len(kt)}_{m}", name=f"qk{len(qt) + len(kt)}_{m}")
                    nc.vector.tensor_scalar_add(sb[:], ps[:], bias[:, m : m + 1])
                    dst.append(sb)

            # V in natural layout [s, dv] with an appended ones column per head:
            # vp[si] is [128, HLOC*65]; head h occupies cols 65h..65h+63, col
            # 65h+64 is all-ones (yields softmax denominators in PV row 64).
            vp = []
            for si in range(ST):
                psv = ppool.tile([128, DH], f32, tag="ps", name="psv")
                for t in range(DT):
                    nc.tensor.matmul(
                        psv[:],
                        lhsT=(xt[t][:, 128 * si : 128 * (si + 1)]),
                        rhs=(wv[t][:, :]),
                        start=(t == 0), stop=False,
                    )
                nc.tensor.matmul(  # + bv broadcast over rows
                    psv[:], lhsT=(ones_row[:]), rhs=(bv_sb[:]), start=False, stop=True,
                )
                v = vpool.tile([128, HLOC * 65], f32r, tag=f"vp{si}", name=f"vp{si}")
                v3 = v[:].rearrange("p (h e) -> p h e", e=65)
                nc.vector.tensor_copy(
                    v3[:, :, 0:64], psv[:].rearrange("p (h d) -> p h d", d=64)
                )
                nc.vector.memset(v3[:, :, 64:65].bitcast(f32), 1.0)
                vp.append(v)

        ctxn = [cpool.tile([128, S], f32r, tag=f"ctxn{m}", name=f"ctxn{m}") for m in range(2)]

        # ---- attention ---------------------------------------------------
        # Head pair (2m, 2m+1) processed together: one [128, 2048] score tile
        # per (pass, k) holds head 2m @ q-half pss (cols 0-1023) and head
        # 2m+1 @ q-half 1-pss (cols 1024-2047), produced by row-packed
        # concurrent K=64 matmuls (lhsT/rhs partition bases 0 and 64).
        # ONE 2048-wide exp per tile (85% ACT efficiency). PV stays M=65
        # (ones-column -> denominators in row 64) into two separate base-0
        # psum ctx tensors. PSUM: 4 (score tile) + 2+2 (ctx pair) = 8 banks.
        with tc.tile_pool(name="pscore", bufs=1, space="PSUM") as spool, \
             tc.tile_pool(name="pctx", bufs=2, space="PSUM") as ctxpool:
            units = [(m, pss, k) for m in range(2) for pss in range(2)
                     for k in range(ST)]
            ctx_cur = {}

            def emit_scores(u):
                m, pss, k = u
                ss = spool.tile([128, 2048], f32, tag="ss", name="ss")
                for half, qhh in ((0, pss), (1, 1 - pss)):
                    qb = half * 64
                    for c2 in range(2):
                        nc.tensor.matmul(
                            ss[:, 1024 * half + 512 * c2 : 1024 * half + 512 * (c2 + 1)],
                            lhsT=kt[m][qb : qb + 64, 128 * k : 128 * (k + 1)],
                            rhs=qt[m][qb : qb + 64,
                                      1024 * qhh + 512 * c2 : 1024 * qhh + 512 * (c2 + 1)],
                            start=True, stop=True,
                        )
                return ss

            ss_cur = emit_scores(units[0])
            for i, u in enumerate(units):
                m, pss, k = u
                if k == 0:
                    ctx_cur[0] = ctxpool.tile([128, 1024], f32, tag="ctx",
                                              name="ctxA")
                    ctx_cur[1] = ctxpool.tile([128, 1024], f32, tag="ctx",
                                              name="ctxB")
                ep = eppool.tile([128, 2048], f32r, tag="ep", name="ep")
                nc.scalar.activation(
                    ep[:], ss_cur[:], Exp, bias=mk_sb[:, k : k + 1], scale=SCALE,
                )
                if i + 1 < len(units):
                    ss_cur = emit_scores(units[i + 1])
                for half in (0, 1):
                    h = 2 * m + half
                    cx = ctx_cur[half]
                    for c2 in range(2):
                        nc.tensor.matmul(
                            cx[0:65, 512 * c2 : 512 * (c2 + 1)],
                            lhsT=vp[k][:, 65 * h : 65 * h + 65],
                            rhs=ep[:, 1024 * half + 512 * c2 : 1024 * half + 512 * (c2 + 1)],
                            start=(k == 0), stop=(k == ST - 1),
                        )
                if k == ST - 1:
                    for half in (0, 1):
                        qhh = pss if half == 0 else 1 - pss
                        qb = half * 64
                        cx = ctx_cur[half]
                        inv = nrmpool.tile([1, 1024], f32, tag="inv", name="inv")
                        nc.vector.reciprocal(inv[:], cx[64:65, :])
                        invb = nrmpool.tile([64, 1024], f32, tag="invb",
                                            name="invb")
                        nc.gpsimd.partition_broadcast(invb[:], inv[:],
                                                      channels=64)
                        nc.vector.tensor_mul(
                            ctxn[m][qb : qb + 64, 1024 * qhh : 1024 * (qhh + 1)],
                            cx[0:64, :], invb[:],
                        )

        # ---- output projection ------------------------------------------
        with tc.tile_pool(name="pout", bufs=2, space="PSUM") as opool:
            for m in range(DT):
                po = opool.tile([128, S], f32, tag="po", name="po")
                for t in range(2):
                    for c in range(NCH):
                        nc.tensor.matmul(
                            po[:, 512 * c : 512 * (c + 1)],
                            lhsT=(wo[t][:, 128 * m : 128 * (m + 1)]),
                            rhs=(ctxn[t][:, 512 * c : 512 * (c + 1)]),
                            start=(t == 0), stop=(t == 1),
                        )
                ob = outpool.tile([128, S], f32, tag="ob", name="ob")
                # ACT is idle after the last exp; alternate engines so
                # consecutive output tiles drain PSUM in parallel.
                if m % 2 == 0:
                    nc.scalar.copy(ob[:], po[:])
                else:
                    nc.vector.tensor_copy(ob[:], po[:])
                nc.sync.dma_start(
                    out=outT[128 * m : 128 * (m + 1), :], in_=ob[:]
                )

    nc.compile()
    return nc


@functools.lru_cache(maxsize=1)
def _compiled():
    return _build()


def _in_maps(x, mask, Wq, bq, Wk, bk, Wv, bv, Wo, bo):
    maps = []
    for c in range(8):
        b, half = c // 2, c % 2
        hs = slice(DH * half, DH * (half + 1))
        maps.append({
            "xT": np.ascontiguousarray(x[b].T),
            "wqT": np.ascontiguousarray(Wq[hs].T),
            "wkT": np.ascontiguousarray(Wk[hs].T),
            "wvT": np.ascontiguousarray(Wv[hs].T),
            "woT": np.ascontiguousarray(Wo[:, hs].T),
            "bq": np.ascontiguousarray(bq[hs]),
            "bk": np.ascontiguousarray(bk[hs]),
            "bv": np.ascontiguousarray(bv[hs]),
            "maskb": np.where(mask[b], 0.0, -1e30).astype(np.float32),
        })
    return maps


def _run(in_maps, trace=False):
    from concourse.bass_utils import run_bass_kernel_spmd

    nc = _compiled()
    return run_bass_kernel_spmd(nc, in_maps, list(range(8)), trace=trace)


def kernel(x, mask, Wq, bq, Wk, bk, Wv, bv, Wo, bo, _trace=False, _res_out=None):
    x = np.asarray(x, dtype=np.float32)
    res = _run(_in_maps(np.asarray(x), np.asarray(mask), np.asarray(Wq),
                        np.asarray(bq), np.asarray(Wk), np.asarray(bk),
                        np.asarray(Wv), np.asarray(bv), np.asarray(Wo),
                        np.asarray(bo)), trace=_trace)
    if _res_out is not None:
        _res_out.append(res)
    out = np.empty((B, S, D), dtype=np.float32)
    for b in range(B):
        pT = res.results[2 * b]["outT"] + res.results[2 * b + 1]["outT"]
        out[b] = pT.T + np.asarray(bo, dtype=np.float32)[None, :]
    return out

